# revision 9
# baseline (speedup 1.0000x reference)
"""Trainium2 Bass kernel for nn_LinearEmbedded (moe_routing).

Reference computation:
    w = weight1[region_ix]             # (B, C, D) gather per-region weights
    out = einsum('abc,bcd->abd', x, w) + bias1[region_ix][None]

Sharding: B (128 regions) split across 8 NeuronCores, 16 per core; the
per-region weight/bias gather happens host-side.

int8 weight path: each w_b row (fixed c) is quantized host-side to int8
with a per-row scale s_bc = max|w_bc|/127 folded into x
(x'_abc = x_abc * s_bc, fp16).  The device converts int8 -> fp16 on DVE
(exact for |q|<=127); the fp16 matmul x' @ q reproduces x @ w with l2 rel
err ~7.4e-3 (resid_var 5.5e-5).  Per-core HBM: w8 4.2 + xt 2.1 + out 2.1
= 8.4 MB (wire floor ~24-25 us at the ~334-358 GB/s per-core HBM rate).

v3 schedule (lessons from v1/v2 traces):
  - each HWDGE dma_start occupies its issuing engine ~0.7 us regardless
    of size, so bulk traffic is batched (w8 2-b pairs, xt 4-b quads, out
    2-b pair stores) -- but b0..b3 w8 and b0 xt stay fine-grained so the
    first converts/matmuls start ~9 us instead of ~12 (v2's mistake).
  - PE cold/warm feedback: the HAM clock gate needs ~3.4 us of sustained
    PE activity to unthrottle 1.2 -> 2.4 GHz and re-throttles after idle.
    5 dummy matmuls bridge the gap between preamble end and b0 readiness;
    deep slots (R_WF=8, 4 w8 pair slots) keep density high after that.
  - PSUM evacuated in 2-b pairs on ScalarE (one ACTIVATE over 2 adjacent
    banks of a single 6-bank psum allocation); last pair split per-b to
    shorten the tail.  Stores ride the SP ring, evac+store share FIFOs.

Engine roles:
    sync   - w8 loads + out stores (HWDGE SP ring), final proof
    vector - int8 -> fp16 converts (one [128,2048] tensor_copy per b)
    tensor - warmup dummies, then 4 K=128 matmuls + K=1 bias matmul per b
    scalar - bias + xt loads (HWDGE ACT ring) + PSUM -> SBUF evacuation

Per-slot DMA semaphores have at most one outstanding transfer each, so
per-slot counting is exact despite out-of-order queue completion.
"""

import numpy as np

A, B, C, D = 128, 128, 512, 512
NCORES = 8
BL = B // NCORES
KC = C // 128
NP = BL // 2   # 2-b pairs
NQ = BL // 4   # xt quads
R_W8P = 4      # w8 pair slots
R_WF = 8       # converted fp16 w slots
R_PB = 6       # psum banks for real work (+1 warmup dummy bank)
N_WARM = 4     # dummy warmup matmuls

_prog = None


def _build_program():
    global _prog
    if _prog is not None:
        return _prog

    import concourse.bass as bass
    import concourse.mybir as mybir
    from contextlib import ExitStack

    F32 = mybir.dt.float32
    F16 = mybir.dt.float16
    I8 = mybir.dt.int8
    nc = bass.Bass("TRN2", target_bir_lowering=False, debug=False)
    # xq rows pack 4 b-slices: [b|b+1|b+2|b+3], each a KC*A=512 f16 chunk
    xq = nc.dram_tensor("xq", [NQ, 128, 4 * KC * A], F16, kind="ExternalInput")
    # w8 rows pack 2 b-slices of [128, KC*D] int8
    w8 = nc.dram_tensor("w8", [NP, 128, 2 * KC * D], I8, kind="ExternalInput")
    bias = nc.dram_tensor("bias", [1, BL * D + A], F16, kind="ExternalInput")
    out = nc.dram_tensor("out", [BL, A, D], F16, kind="ExternalOutput")

    WB = KC * D  # 2048 int8 per b per partition

    ctx = ExitStack()
    with ctx:
        xqs = [
            ctx.enter_context(nc.sbuf_tensor(f"xqs{i}", [128, 4 * KC * A], F16))
            for i in range(2)
        ]
        w8s = [
            ctx.enter_context(nc.sbuf_tensor(f"w8s{i}", [128, 2 * WB], I8))
            for i in range(R_W8P)
        ]
        wfs = [
            ctx.enter_context(nc.sbuf_tensor(f"wfs{i}", [128, WB], F16))
            for i in range(R_WF)
        ]
        ots = [
            ctx.enter_context(nc.sbuf_tensor(f"ots{i}", [128, 2 * D], F16))
            for i in range(2)
        ]
        bias_t = ctx.enter_context(nc.sbuf_tensor("bias_t", [1, BL * D + A], F16))
        psum = ctx.enter_context(nc.psum_tensor("psum", [A, R_PB * D], F32))
        psum_d = ctx.enter_context(nc.psum_tensor("psum_d", [A, D], F32))

        # single-transfer sems (one outstanding each)
        s_w8a = ctx.enter_context(nc.semaphore("s_w8a"))  # b0 low half
        s_w8b = ctx.enter_context(nc.semaphore("s_w8b"))  # b0 high half
        s_w8c = ctx.enter_context(nc.semaphore("s_w8c"))  # b1
        s_w8d = ctx.enter_context(nc.semaphore("s_w8d"))  # b2
        s_w8e = ctx.enter_context(nc.semaphore("s_w8e"))  # b3
        s_wp = [ctx.enter_context(nc.semaphore(f"s_wp{i}")) for i in range(R_W8P)]
        s_xa = ctx.enter_context(nc.semaphore("s_xa"))  # b0 xt
        s_xb = ctx.enter_context(nc.semaphore("s_xb"))  # b1-b3 xt
        s_xq = [ctx.enter_context(nc.semaphore(f"s_xq{i}")) for i in range(2)]
        s_op = [ctx.enter_context(nc.semaphore(f"s_op{i}")) for i in range(2)]
        s_wf = ctx.enter_context(nc.semaphore("s_wf"))  # 2 units per b
        s_b = ctx.enter_context(nc.semaphore("s_b"))
        s_pe = ctx.enter_context(nc.semaphore("s_pe"))  # 1 per b
        s_cp = ctx.enter_context(nc.semaphore("s_cp"))  # 1 per evac op
        s_done = ctx.enter_context(nc.semaphore("s_done"))

        sync, scalar, tensor, vector = nc.sync, nc.scalar, nc.tensor, nc.vector

        # --- SP engine: w8 loads only (no head-of-line stalls), then proof ---
        if True:
            # fine-grained fill: b0 halves, b1, b2, b3
            sync.dma_start(w8s[0][:, 0 : WB // 2], w8[0, :, 0 : WB // 2]).then_inc(
                s_w8a, 16
            )
            sync.dma_start(w8s[0][:, WB // 2 : WB], w8[0, :, WB // 2 : WB]).then_inc(
                s_w8b, 16
            )
            sync.dma_start(w8s[0][:, WB : 2 * WB], w8[0, :, WB : 2 * WB]).then_inc(
                s_w8c, 16
            )
            sync.dma_start(w8s[1][:, 0:WB], w8[1, :, 0:WB]).then_inc(s_w8d, 16)
            sync.dma_start(w8s[1][:, WB : 2 * WB], w8[1, :, WB : 2 * WB]).then_inc(
                s_w8e, 16
            )

            for p in range(2, NP):  # pairs b=2p,2p+1 into slot p % R_W8P
                slot = p % R_W8P
                if p >= 4:
                    # slot held b-singles (p<2 region) or pair p - R_W8P
                    sync.wait_ge(s_wf, 4 * (p - R_W8P) + 4)
                sync.dma_start(w8s[slot][:], w8[p, :, :]).then_inc(s_wp[slot], 16)

            # tail: prove everything landed
            sync.wait_ge(s_pe, BL)
            sync.wait_ge(s_cp, 9)
            sync.wait_ge(s_wf, 2 * BL)
            sync.wait_ge(s_b, 16)
            for s in (s_w8a, s_w8b, s_w8c, s_w8d, s_w8e):
                sync.wait_ge(s, 16)
            sync.wait_ge(s_wp[0], 16)   # pair 4
            sync.wait_ge(s_wp[1], 16)   # pair 5
            sync.wait_ge(s_wp[2], 32)   # pairs 2, 6
            sync.wait_ge(s_wp[3], 32)   # pairs 3, 7
            sync.wait_ge(s_xa, 16)
            sync.wait_ge(s_xb, 16)
            sync.wait_ge(s_xq[0], 16)   # quad 2
            sync.wait_ge(s_xq[1], 32)   # quads 1, 3
            sync.wait_ge(s_op[0], 80)   # pairs 0,2,4,6 + b15
            sync.wait_ge(s_op[1], 64)   # pairs 1,3,5 + b14
            sync.wait_ge(s_done, 3)

        # --- DVE engine: int8 -> fp16 weight converts ---
        if True:
            for b in range(BL):
                p, half = b // 2, b % 2
                fslot = b % R_WF
                if b >= R_WF:
                    vector.wait_ge(s_pe, b - R_WF + 1)
                if b == 0:
                    vector.wait_ge(s_w8a, 16)
                    nc.vector.tensor_copy(
                        wfs[0][:, 0 : WB // 2], w8s[0][:, 0 : WB // 2]
                    ).then_inc(s_wf, 1)
                    vector.wait_ge(s_w8b, 16)
                    nc.vector.tensor_copy(
                        wfs[0][:, WB // 2 : WB], w8s[0][:, WB // 2 : WB]
                    ).then_inc(s_wf, 1)
                    continue
                if b == 1:
                    vector.wait_ge(s_w8c, 16)
                elif b == 2:
                    vector.wait_ge(s_w8d, 16)
                elif b == 3:
                    vector.wait_ge(s_w8e, 16)
                else:
                    vector.wait_ge(s_wp[p % R_W8P], 16 * ((p - 2) // R_W8P + 1))
                slot = p % R_W8P if b >= 4 else b // 2
                nc.vector.tensor_copy(
                    wfs[fslot][:], w8s[slot][:, half * WB : (half + 1) * WB]
                ).then_inc(s_wf, 2)
            vector.sem_inc(s_done, 1)

        # --- PE engine: warmup dummies + per-b matmuls ---
        if True:
            ones = bias_t[:, BL * D : BL * D + A]
            for i in range(N_WARM):
                nc.tensor.matmul(
                    psum_d[:], xqs[0][:, 0:A], wfs[0][:, 0:D], start=True, stop=True
                )
            for b in range(BL):
                q, fslot = b // 4, b % R_WF
                if b >= R_PB:
                    tensor.wait_ge(s_cp, (b - R_PB) // 2 + 1)
                if b == 0:
                    tensor.wait_ge(s_xa, 16)
                elif b == 1:
                    tensor.wait_ge(s_xb, 16)
                elif b % 4 == 0:
                    tensor.wait_ge(s_xq[q % 2], 16 * ((q - 1) // 2 + 1))
                for k in range(KC):
                    if k == 0:
                        tensor.wait_ge(s_wf, 2 * b + 1)
                    elif k == 2:
                        tensor.wait_ge(s_wf, 2 * b + 2)
                    nc.tensor.matmul(
                        psum[:, (b % R_PB) * D : (b % R_PB) * D + D],
                        xqs[q % 2][
                            :,
                            (b % 4) * KC * A + k * A : (b % 4) * KC * A + (k + 1) * A,
                        ],
                        wfs[fslot][:, k * D : (k + 1) * D],
                        start=(k == 0),
                        stop=False,
                    )
                if b == 0:
                    tensor.wait_ge(s_b, 16)
                nc.tensor.matmul(
                    psum[:, (b % R_PB) * D : (b % R_PB) * D + D],
                    ones,
                    bias_t[:, b * D : (b + 1) * D],
                    start=False,
                    stop=True,
                ).then_inc(s_pe, 1)
            tensor.sem_inc(s_done, 1)

        # --- ACT engine: bias + xt loads + PSUM evac + out stores ---
        if True:
            scalar.dma_start(bias_t[:], bias[:]).then_inc(s_b, 16)
            scalar.dma_start(xqs[0][:, 0 : KC * A], xq[0, :, 0 : KC * A]).then_inc(
                s_xa, 16
            )
            scalar.dma_start(
                xqs[0][:, KC * A : 4 * KC * A], xq[0, :, KC * A : 4 * KC * A]
            ).then_inc(s_xb, 16)
            scalar.dma_start(xqs[1][:], xq[1, :, :]).then_inc(s_xq[1], 16)
            for p in range(7):
                oslot = p % 2
                if p >= 2:
                    scalar.wait_ge(s_op[oslot], 16 * ((p - 2) // 2 + 1))
                scalar.wait_ge(s_pe, 2 * p + 2)
                nc.scalar.copy(
                    ots[oslot][:],
                    psum[:, (2 * p % R_PB) * D : (2 * p % R_PB) * D + 2 * D],
                ).then_inc(s_cp, 1)
                # same FIFO: store issues only after the copy completed
                scalar.dma_start(
                    out[2 * p : 2 * p + 2, :, :], ots[oslot][:]
                ).then_inc(s_op[oslot], 16)
                if p == 1:  # s_pe >= 4 held: quad 0 consumed
                    scalar.dma_start(xqs[0][:], xq[2, :, :]).then_inc(s_xq[0], 16)
                elif p == 3:  # s_pe >= 8 held: quad 1 consumed
                    scalar.dma_start(xqs[1][:], xq[3, :, :]).then_inc(s_xq[1], 16)
            # last pair split per-b: b14 -> ots[1], b15 -> ots[0]
            scalar.wait_ge(s_op[1], 48)
            scalar.wait_ge(s_pe, 15)
            nc.scalar.copy(
                ots[1][:, 0:D], psum[:, (14 % R_PB) * D : (14 % R_PB) * D + D]
            ).then_inc(s_cp, 1)
            scalar.dma_start(out[14, :, :], ots[1][:, 0:D]).then_inc(s_op[1], 16)
            scalar.wait_ge(s_op[0], 64)
            scalar.wait_ge(s_pe, 16)
            nc.scalar.copy(
                ots[0][:, 0:D], psum[:, (15 % R_PB) * D : (15 % R_PB) * D + D]
            ).then_inc(s_cp, 1)
            scalar.dma_start(out[15, :, :], ots[0][:, 0:D]).then_inc(s_op[0], 16)
            scalar.sem_inc(s_done, 1)

        # No Block: engine streams end bare; completion proven by SP waits.

    _prog = nc
    return nc


def _shard_inputs(x, region_ix, weight1, bias1):
    in_maps = []
    for c in range(NCORES):
        bs = slice(c * BL, (c + 1) * BL)
        rloc = region_ix[bs]
        wg = weight1[rloc]  # (BL, C, D) f32
        # per-row int8 quantization; scale folded into x below
        s = np.maximum(np.abs(wg).max(axis=2), 1e-30) / 127.0  # (BL, C)
        q = np.clip(np.rint(wg / s[:, :, None]), -127, 127).astype(np.int8)
        wdev = np.ascontiguousarray(
            q.reshape(BL, KC, 128, D).transpose(0, 2, 1, 3)
        ).reshape(BL, 128, KC * D)
        w8v = np.ascontiguousarray(
            wdev.reshape(NP, 2, 128, KC * D).transpose(0, 2, 1, 3)
        ).reshape(NP, 128, 2 * KC * D)
        xs = (x[:, bs, :] * s[None, :, :]).astype(np.float16)  # (A, BL, C)
        xsv = np.ascontiguousarray(xs.transpose(1, 2, 0))  # (BL, C, A)
        xtv = np.ascontiguousarray(
            xsv.reshape(BL, KC, 128, A).transpose(0, 2, 1, 3)
        ).reshape(BL, 128, KC * A)
        xqv = np.ascontiguousarray(
            xtv.reshape(NQ, 4, 128, KC * A).transpose(0, 2, 1, 3)
        ).reshape(NQ, 128, 4 * KC * A)
        bg = np.concatenate(
            [bias1[rloc].astype(np.float16).reshape(BL * D), np.ones(A, np.float16)]
        ).reshape(1, BL * D + A)
        in_maps.append({"xq": xqv, "w8": w8v, "bias": bg})
    return in_maps


def kernel(x, region_ix, weight1, bias1):
    from concourse.bass_utils import run_bass_kernel_spmd

    x = np.asarray(x, dtype=np.float32)
    region_ix = np.asarray(region_ix).astype(np.int64)
    weight1 = np.asarray(weight1, dtype=np.float32)
    bias1 = np.asarray(bias1, dtype=np.float32)

    nc = _build_program()
    in_maps = _shard_inputs(x, region_ix, weight1, bias1)
    res = run_bass_kernel_spmd(nc, in_maps, core_ids=list(range(NCORES)))

    outv = np.empty((A, B, D), dtype=np.float32)
    for c in range(NCORES):
        outv[:, c * BL : (c + 1) * BL, :] = res.results[c]["out"].transpose(1, 0, 2)
    return outv
